# revision 1
# baseline (speedup 1.0000x reference)
"""Block-sparse attention (nn_BlockSparseAttention) on 8 TRN2 NeuronCores.

Strategy: head-parallel (16 heads / 8 cores = 2 heads per core).
Per core, all in bf16 on the TensorEngine with f32 PSUM accumulation:
  1. QKV projections in transposed layout (QT/KT/VT = W^T @ hidden^T),
     q pre-scaled by 1/sqrt(d) on the host. RoPE applied after an ACT
     PSUM->bf16 eviction with three 4x-mode DVE multiplies + one add
     against bf16 cos/sin tables (partition-shifted reads implement
     rotate_half).
  2. Attention with scores in [keys, q] orientation so PV needs no
     transposes. Softmax without max-subtraction (scores stay in f32
     exp range). The reference's mask semantics (masked scores = 0 =>
     probs contribution exp(0) = 1) are implemented exactly via a
     decomposition: exp() runs UNMASKED straight out of PSUM on the
     Scalar engine, a token-resolution bf16 binary mask (materialized
     on the host, streamed over the idle DVE DMA queue) zeroes
     unselected blocks in a 4x-mode scalar_tensor_tensor, and the
     "+1 per masked key" contributions are restored with two tiny
     extra matmuls per chunk (block-sums of V x complement mask, and
     64 x complement-mask count for the denominator) accumulated into
     the same PSUM groups. Normalize = single tensor divide.
     Attention groups are software-pipelined: scores/exp of group g
     interleave with PV/den matmuls of group g-1 so the Scalar engine
     never stalls the PE.
  3. One AllGather per 512-col sequence chunk (4 total); ALL o_proj
     matmuls are deferred to the end of the kernel so the last chunk's
     AllGather latency hides under o_proj of the earlier chunks.
Host side: input rearrangement/casting, top-k block mask (+ its
token-resolution expansion), RoPE tables, and final concat+transpose
of the 8 output shards.
"""
import sys

if "/opt/trn_rl_repo" not in sys.path:
    sys.path.insert(0, "/opt/trn_rl_repo")

import numpy as np
import ml_dtypes

import concourse.bass as bass
import concourse.tile as tile
import concourse.mybir as mybir
from concourse import bacc
from concourse.bass_utils import run_bass_kernel_spmd
from concourse.masks import make_identity

# problem constants (hardcoded per harness contract)
B, S, HID = 1, 2048, 2048
NH, HD, BS = 16, 128, 64
RATIO = 0.5
THETA = 10000.0
NCORES = 8
HPC = NH // NCORES          # heads per core = 2
P = 128                     # partitions
CH = HID // P               # contraction chunks = 16
KT = S // P                 # key tiles = 16
FB = 512                    # free-dim block (psum bank)
QC = S // FB                # q chunks = 4
NQB = S // BS               # 32 blocks per side
QB_PER_FB = FB // BS        # 8 q-blocks per 512 chunk
KTB = 2                     # key tiles per scores psum tile
CSUB = 4                    # hidden^T contraction subtiles per chunk
WSUB = 4                    # weight DMA subtiles (startup latency)

BF = mybir.dt.bfloat16
F32 = mybir.dt.float32
MUL = mybir.AluOpType.mult
ADD = mybir.AluOpType.add

_CACHE = {}


def _build():
    nc = bacc.Bacc("TRN2", target_bir_lowering=False, debug=False,
                   num_devices=NCORES)

    hT = nc.dram_tensor("hT", [QC, P, CH, FB], BF, kind="ExternalInput").ap()
    wq = nc.dram_tensor("wq", [HPC, P, CH, P], BF, kind="ExternalInput").ap()
    wk = nc.dram_tensor("wk", [HPC, P, CH, P], BF, kind="ExternalInput").ap()
    wv = nc.dram_tensor("wv", [HPC, P, CH, P], BF, kind="ExternalInput").ap()
    wo = nc.dram_tensor("wo", [P, CH, HPC * P], BF, kind="ExternalInput").ap()
    cosT = nc.dram_tensor("cosT", [P, S], BF, kind="ExternalInput").ap()
    sinT = nc.dram_tensor("sinT", [P, S], BF, kind="ExternalInput").ap()  # pre-signed
    binF = nc.dram_tensor("binF", [QC, HPC, P, KT, FB], BF,
                          kind="ExternalInput").ap()
    binN = nc.dram_tensor("binN", [NQB, HPC, NQB], BF, kind="ExternalInput").ap()
    out = nc.dram_tensor("out", [HPC * P, S], F32, kind="ExternalOutput").ap()

    with tile.TileContext(nc) as tc:
        with (
            tc.tile_pool(name="cp", bufs=1) as cp,          # persistent tensors
            tc.tile_pool(name="pp", bufs=1, space="PSUM") as pp,
            tc.tile_pool(name="dp", bufs=1, space="DRAM") as dp,
        ):
            QTr = cp.tile([P, HPC, S], BF, name="QTr")
            KTr = cp.tile([P, HPC, S], BF, name="KTr")
            V_sbs = [cp.tile([P, KT, P], BF, name=f"V_h{h}")
                     for h in range(HPC)]
            corrT_sb = cp.tile([NQB, HPC, P], BF, name="corrT_sb")
            # chunk-0 token-resolution masks live in the persistent pool so
            # their DMAs can run during the QKV phase without pool aliasing
            binF0 = [cp.tile([P, KT, FB], BF, name=f"binF0_{h}")
                     for h in range(HPC)]

            # ---------------- QKV + RoPE (phase-scoped pool) ----------------
            qp = tc.alloc_tile_pool(name="qp", bufs=2)

            # a tiny warm-up AllGather right behind the kernel-entry barrier
            # absorbs the first-collective cost (~20us extra on the first
            # real AllGather otherwise)
            cc_warm_in = dp.tile([P, 8], BF, name="cc_warm_in")
            cc_warm_out = dp.tile([NCORES * P, 8], BF, name="cc_warm_out",
                                  addr_space="Shared")
            nc.gpsimd.collective_compute(
                "AllGather",
                mybir.AluOpType.bypass,
                replica_groups=[list(range(NCORES))],
                ins=[cc_warm_in.opt()],
                outs=[cc_warm_out.opt()],
            )

            # Startup DMA choreography across three queues: the first QKV
            # group alone needs all four qc0 hidden subtiles (c spans the
            # full hidden dim) plus its weights — one queue cannot feed the
            # PE fast enough. Each queue's issue order matches consumption.
            CW = CH // CSUB
            w_drams = {"k": wk, "v": wv, "q": wq}
            CSW = CH // WSUB
            w_sbs = {}
            for h in range(HPC):
                for proj in ("k", "v", "q"):
                    w_sbs[(h, proj)] = qp.tile(
                        [P, CH, P], BF, name=f"w_{h}{proj}",
                        tag="w_sb", bufs=6)
            hT_sbs = [[qp.tile([P, CW, FB], BF, name=f"hT_c{qcb}_{cs}",
                               bufs=1) for cs in range(CSUB)]
                      for qcb in range(QC)]
            cos_sb = qp.tile([P, S], BF, name="cos_sb", bufs=1)
            sin_sb = qp.tile([P, S], BF, name="sin_sb", bufs=1)

            def dma_w(eng, h, proj, sbi):
                csl = slice(sbi * CSW, (sbi + 1) * CSW)
                eng.dma_start(w_sbs[(h, proj)][:, csl, :],
                              w_drams[proj][h, :, csl, :])

            def dma_hT(eng, qcb, cs):
                eng.dma_start(hT_sbs[qcb][cs][:],
                              hT[qcb, :, cs * CW:(cs + 1) * CW, :])

            # sync queue: all 16 hidden^T subtiles, qc-major (the QKV loop
            # consumes them in this order)
            for qcb in range(QC):
                for cs in range(CSUB):
                    dma_hT(nc.sync, qcb, cs)
            # scalar: head-0 weights in 128KB chunks (first matmul starts
            # ~3us in), cos right behind the k weights for the first RoPE
            # eviction; gpsimd: sin + head-1 weights on the SWDGE path
            nc.gpsimd.dma_start(sin_sb[:], sinT[:])
            for h in range(HPC):
                eng = nc.scalar if h == 0 else nc.gpsimd
                for proj in ("k", "v", "q"):
                    for sbi in range(WSUB):
                        dma_w(eng, h, proj, sbi)
                    if h == 0 and proj == "k":
                        nc.scalar.dma_start(cos_sb[:], cosT[:])

            binN_sb = cp.tile([NQB, HPC, NQB], BF, name="binN_sb")
            nc.gpsimd.dma_start(binN_sb[:], binN[:])
            wo_sb = cp.tile([P, CH, HPC * P], BF, name="wo_sb")
            nc.gpsimd.dma_start(wo_sb[:], wo[:])
            ones_sb = cp.tile([P, P], BF, name="ones_sb")
            nc.vector.memset(ones_sb[:], 1.0)
            c64_sb = cp.tile([NQB, P], BF, name="c64_sb")
            nc.vector.memset(c64_sb[:], float(BS))
            ident = cp.tile([P, P], BF, name="ident")
            make_identity(nc, ident[:])
            # warm the ACT exp table during the QKV phase so the first
            # attention group doesn't pay the 1.3us table load
            warm = cp.tile([1, 2], F32, name="warm")
            nc.vector.memset(warm[:], 0.0)
            nc.scalar.activation(out=warm[0:1, 0:1], in_=warm[0:1, 1:2],
                                 func=mybir.ActivationFunctionType.Exp)
            # chunk-0 masks on the scalar queue behind the weights
            for h in range(HPC):
                nc.scalar.dma_start(binF0[h][:], binF[0, h])

            def rope_evict(ps, dst, h, qsl):
                # dst = ps*cos + rotate_half(ps)*sin, all bf16 after an ACT
                # eviction; sin is pre-signed so both halves are multiplies
                qbf = qp.tile([P, FB], BF, name="qbf", tag="qbf", bufs=3)
                nc.scalar.copy(out=qbf[:], in_=ps[:])
                tcos = qp.tile([P, FB], BF, name="tcos", tag="tcos", bufs=2)
                nc.vector.tensor_mul(
                    out=tcos[:], in0=qbf[:], in1=cos_sb[:, qsl])
                # partition-shifted reads (rotate_half) must come from PSUM:
                # the DVE rejects two SBUF inputs on different base partitions
                tsin = qp.tile([P, FB], BF, name="tsin", tag="tsin", bufs=2)
                nc.vector.tensor_mul(
                    out=tsin[0:64, :], in0=ps[64:128, :],
                    in1=sin_sb[0:64, qsl])
                nc.vector.tensor_mul(
                    out=tsin[64:128, :], in0=ps[0:64, :],
                    in1=sin_sb[64:128, qsl])
                nc.vector.tensor_add(
                    out=dst[:, h, qsl], in0=tcos[:], in1=tsin[:])

            # K and V for BOTH heads first (attention group 0 needs the full
            # K and V of head 0, including the last sequence chunk's keys —
            # a q-first order would leave the first PV waiting on the V
            # transposes); all Q projections follow.
            vT_sbs = {}

            def qkv_group(qc, h, proj):
                qsl = slice(qc * FB, (qc + 1) * FB)
                w_sb = w_sbs[(h, proj)]
                ps = pp.tile([P, FB], F32, name="ps_acc",
                             tag="ps_acc", bufs=2)
                for c in range(CH):
                    nc.tensor.matmul(
                        ps[:],
                        lhsT=w_sb[:, c, :],
                        rhs=hT_sbs[qc][c // CW][:, c % CW, :],
                        start=(c == 0),
                        stop=(c == CH - 1),
                    )
                if proj != "v":
                    rope_evict(ps, KTr if proj == "k" else QTr, h, qsl)
                else:
                    vT_c = qp.tile([P, FB], BF, name="vT_c",
                                   tag="vT_c", bufs=8)
                    nc.scalar.copy(out=vT_c[:], in_=ps[:])
                    vT_sbs[(h, qc)] = vT_c
                    # V natural layout via PE transposes. NOT DMA transposes:
                    # the tile framework serializes DMA transposes against
                    # collectives, which cross-blocks the AllGathers and (via
                    # the in-order ACT queue) the RoPE evictions.
                    ps_v = pp.tile([P, QC, P], BF, name="ps_v",
                                   tag="ps_s", bufs=2)
                    with nc.allow_low_precision(reason="bf16 V transpose"):
                        for j in range(QC):
                            nc.tensor.transpose(
                                ps_v[:, j, :], vT_c[:, j * P:(j + 1) * P],
                                ident[:])
                    nc.scalar.copy(
                        out=V_sbs[h][:, qc * QC:(qc + 1) * QC, :],
                        in_=ps_v[:])

            for qc in range(QC):
                for h in range(HPC):
                    for proj in ("k", "v"):
                        qkv_group(qc, h, proj)
            for qc in range(QC):
                for h in range(HPC):
                    qkv_group(qc, h, "q")

            # block-sums of V^T -> [d, kb] -> transpose -> corrT [kb, d]
            for h in range(HPC):
                bsum = qp.tile([P, NQB], BF, name="bsum", tag="bsum")
                with nc.allow_low_precision(
                        reason="block-sum correction term, 64-wide bf16 sum"):
                    for qc in range(QC):
                        nc.vector.tensor_reduce(
                            out=bsum[:, qc * QB_PER_FB:(qc + 1) * QB_PER_FB],
                            in_=vT_sbs[(h, qc)].rearrange(
                                "p (b e) -> p b e", e=BS),
                            axis=mybir.AxisListType.X,
                            op=mybir.AluOpType.add,
                        )
                ps_t = pp.tile([NQB, P], BF, name="ps_t", tag="ps_s",
                               bufs=2)
                with nc.allow_low_precision(
                        reason="block-sum correction term, 64-wide bf16 sum"):
                    nc.tensor.transpose(ps_t[:], bsum[:], ident[:])
                nc.scalar.copy(out=corrT_sb[:, h, :], in_=ps_t[:])

            qp.release()

            # ------------- attention (pipelined) + AllGather -------------
            wp = tc.alloc_tile_pool(name="wp", bufs=2)
            # sequence chunks gathered in PAIRS: single-chunk gathers
            # measured ~25-37us EACH on the serial CC stream (latency-
            # dominated), pairs ~26-31 for twice the payload — two ops
            # keep the stream off the critical path
            cc_ins = {pr: dp.tile([HPC * P, 2 * FB], BF, name=f"cc_in{pr}")
                      for pr in range(QC // 2)}
            cc_outs = {pr: dp.tile([NCORES * HPC * P, 2 * FB], BF,
                                   name=f"cc_out{pr}", addr_space="Shared")
                       for pr in range(QC // 2)}

            # stream the remaining token-resolution masks, split across the
            # scalar and gpsimd queues, all issued up front: their transfers
            # serialize behind the qp-pool alias release (~end of QKV) and
            # land one-per-12us, comfortably ahead of their groups. The
            # o_proj gather tiles share the buffers (same shape, disjoint
            # live ranges) to stay inside SBUF.
            binF_wp = {}
            pending = [(qc, h) for qc in (1, 2, 3) for h in range(HPC)]

            def issue_binF():
                # gpsimd only: a trigger on the scalar/ACT queue would
                # head-block the attention exps while it waits for the
                # qp-pool alias release
                if pending:
                    qc, h = pending.pop(0)
                    t = wp.tile([P, KT, FB], BF, name=f"binF_{qc}{h}",
                                tag="binF", bufs=4)
                    nc.gpsimd.dma_start(t[:], binF[qc, h])
                    binF_wp[(qc, h)] = t

            for _ in range(2):
                issue_binF()

            def emit_pv_block(st):
                # PV + correction for the previous (h, qc): fills PE time
                # while the Scalar engine exps the current group's scores
                h, qc, pts = st["h"], st["qc"], st["pts"]
                ps_o = pp.tile([P, FB], F32, name="ps_o", tag="ps_o", bufs=1)
                st["ps_o"] = ps_o
                for kt in range(KT):
                    nc.tensor.matmul(
                        ps_o[:],
                        lhsT=V_sbs[h][:, kt, :],
                        rhs=pts[kt // KTB][:, kt % KTB, :],
                        start=(kt == 0), stop=False,
                    )
                binN_ap = binN_sb[:, h,
                                  qc * QB_PER_FB:(qc + 1) * QB_PER_FB]
                nc.tensor.matmul(
                    ps_o[:],
                    lhsT=corrT_sb[:, h, :],
                    rhs=binN_ap[:, :, None].to_broadcast(
                        [NQB, QB_PER_FB, BS]),
                    start=False, stop=True,
                )

            def emit_den_block(st):
                h, qc, pts = st["h"], st["qc"], st["pts"]
                ps_d = pp.tile([P, FB], F32, name="ps_d", tag="ps_d", bufs=1)
                for kt in range(KT):
                    nc.tensor.matmul(
                        ps_d[:],
                        lhsT=ones_sb[:],
                        rhs=pts[kt // KTB][:, kt % KTB, :],
                        start=(kt == 0), stop=False,
                    )
                binN_ap = binN_sb[:, h,
                                  qc * QB_PER_FB:(qc + 1) * QB_PER_FB]
                nc.tensor.matmul(
                    ps_d[:],
                    lhsT=c64_sb[:],
                    rhs=binN_ap[:, :, None].to_broadcast(
                        [NQB, QB_PER_FB, BS]),
                    start=False, stop=True,
                )
                rden = wp.tile([P, FB], F32, name="rden", tag="rden")
                nc.vector.reciprocal_approx_fast(out=rden[:], in_=ps_d[:])
                at_c = wp.tile([P, FB], BF, name="at_c", tag="at_c", bufs=4)
                nc.vector.tensor_mul(out=at_c[:], in0=st["ps_o"][:],
                                     in1=rden[:])
                half = slice((qc % 2) * FB, (qc % 2 + 1) * FB)
                nc.sync.dma_start(
                    cc_ins[qc // 2][h * P:(h + 1) * P, half], at_c[:])
                if h == HPC - 1 and qc % 2 == 1:
                    nc.gpsimd.collective_compute(
                        "AllGather",
                        mybir.AluOpType.bypass,
                        replica_groups=[list(range(NCORES))],
                        ins=[cc_ins[qc // 2].opt()],
                        outs=[cc_outs[qc // 2].opt()],
                    )

            prev = None
            for qc in range(QC):
                qsl = slice(qc * FB, (qc + 1) * FB)
                for h in range(HPC):
                    binF_t = binF0[h] if qc == 0 else binF_wp[(qc, h)]
                    pts = []

                    def emit_scores(ktp, h=h, qsl=qsl, binF_t=binF_t,
                                    pts=pts):
                        ps_s = pp.tile([P, KTB, FB], F32, name="ps_s",
                                       tag="ps_s", bufs=2)
                        for j in range(KTB):
                            kt = KTB * ktp + j
                            nc.tensor.matmul(
                                ps_s[:, j, :],
                                lhsT=KTr[:, h, kt * P:(kt + 1) * P],
                                rhs=QTr[:, h, qsl],
                                start=True, stop=True,
                            )
                        pt = wp.tile([P, KTB, FB], BF, name="probsT",
                                     tag="probsT", bufs=2 * (KT // KTB))
                        nc.scalar.activation(
                            out=pt[:], in_=ps_s[:],
                            func=mybir.ActivationFunctionType.Exp)
                        nc.vector.tensor_mul(
                            out=pt[:], in0=pt[:],
                            in1=binF_t[:, KTB * ktp:KTB * (ktp + 1), :])
                        pts.append(pt)

                    # software pipeline: scores/exp of this group interleave
                    # with PV/den matmuls of the previous group
                    emit_scores(0)
                    emit_scores(1)
                    if prev is not None:
                        emit_pv_block(prev)
                    emit_scores(2)
                    emit_scores(3)
                    if prev is not None:
                        emit_den_block(prev)
                    for ktp in range(4, KT // KTB):
                        emit_scores(ktp)
                    issue_binF()
                    prev = {"h": h, "qc": qc, "pts": pts}
            emit_pv_block(prev)
            emit_den_block(prev)

            # ---------------- o_proj, all chunks deferred ----------------
            # the last chunk's AllGather latency hides under o_proj of the
            # earlier chunks (their AllGathers completed mid-attention)
            def emit_oproj(qc):
                ag_sb = wp.tile([P, CH, FB], BF, name="ag_sb", tag="binF",
                                bufs=4)
                nc.sync.dma_start(
                    ag_sb[:],
                    cc_outs[qc // 2].rearrange("(c p) s -> p c s", p=P)
                    [:, :, (qc % 2) * FB:(qc % 2 + 1) * FB])
                for strip in range(HPC):
                    ssl = slice(strip * P, (strip + 1) * P)
                    ps_w = pp.tile([P, FB], F32, name="ps_w", tag="ps_acc",
                                   bufs=2)
                    for c in range(CH):
                        nc.tensor.matmul(
                            ps_w[:],
                            lhsT=wo_sb[:, c, ssl],
                            rhs=ag_sb[:, c, :],
                            start=(c == 0),
                            stop=(c == CH - 1),
                        )
                    ot = wp.tile([P, FB], F32, name="ot", tag="ot")
                    nc.vector.tensor_copy(out=ot[:], in_=ps_w[:])
                    nc.sync.dma_start(
                        out[strip * P:(strip + 1) * P, qc * FB:(qc + 1) * FB],
                        ot[:],
                    )

            for qc in range(QC):
                emit_oproj(qc)
            wp.release()

    nc.compile()
    return nc


def _host_prep(hidden_states, q_w, k_w, v_w, o_w, sparsity_pattern):
    hs = np.asarray(hidden_states, dtype=np.float32).reshape(S, HID)
    qw = np.asarray(q_w, dtype=np.float32)
    kw = np.asarray(k_w, dtype=np.float32)
    vw = np.asarray(v_w, dtype=np.float32)
    ow = np.asarray(o_w, dtype=np.float32)
    sp = np.asarray(sparsity_pattern, dtype=np.float32)

    bf = ml_dtypes.bfloat16

    # hidden^T -> [qcb, p, c, s'] (s-chunk-major so chunk DMAs are contiguous)
    hT = np.ascontiguousarray(
        hs.T.reshape(CH, P, QC, FB).transpose(2, 1, 0, 3)).astype(bf)

    # block mask with per-head top-k threshold
    kk = max(1, int(NH * NQB * NQB * RATIO / NH))
    flat = sp.reshape(NH, -1)
    th = np.partition(flat, -kk, axis=1)[:, -kk]
    bm = (sp > th[:, None, None]).astype(np.float32)  # [NH, 32 qb, 32 kb]

    # RoPE tables in [d, s] layout (bf16); sin pre-signed for rotate_half
    inv = 1.0 / (THETA ** (np.arange(0, HD, 2, dtype=np.float32) / HD))
    fr = np.arange(S, dtype=np.float32)[:, None] * inv[None, :]  # [S, 64]
    embT = np.ascontiguousarray(np.concatenate([fr, fr], axis=1).T)  # [128,S]
    cosT = np.cos(embT)
    sinT = np.sin(embT)
    sinT[:64] *= -1.0
    cosT = cosT.astype(bf)
    sinT = sinT.astype(bf)

    def w_per_head(w, h, scale=1.0):
        # [HID, 128] -> [p, c, d]
        return np.ascontiguousarray(
            (w[:, h * HD:(h + 1) * HD] * scale)
            .reshape(CH, P, HD).transpose(1, 0, 2))

    def tok_mask(h):
        # [QC, P, KT, FB]: token-resolution mask in the kernel's
        # [k-partition, k-tile, q] orientation
        tok = np.repeat(np.repeat(bm[h].T, BS, axis=0), BS, axis=1)
        return tok.reshape(KT, P, QC, FB).transpose(2, 1, 0, 3)

    qscale = 1.0 / np.sqrt(HD)
    in_maps = []
    for r in range(NCORES):
        heads = [HPC * r + i for i in range(HPC)]
        wq_r = np.stack([w_per_head(qw, h, qscale) for h in heads]).astype(bf)
        wk_r = np.stack([w_per_head(kw, h) for h in heads]).astype(bf)
        wv_r = np.stack([w_per_head(vw, h) for h in heads]).astype(bf)
        wo_r = np.ascontiguousarray(
            ow[:, r * HPC * HD:(r + 1) * HPC * HD]
            .reshape(CH, P, HPC * HD).transpose(1, 0, 2)).astype(bf)
        binF_r = np.stack([tok_mask(h) for h in heads], axis=1)  # [QC,HPC,...]
        # complement mask [kb, h, qb] for the masked-block corrections
        mN = np.stack([1.0 - bm[h].T for h in heads], axis=1)  # [32, HPC, 32]
        in_maps.append({
            "hT": hT,
            "wq": wq_r, "wk": wk_r, "wv": wv_r, "wo": wo_r,
            "cosT": cosT, "sinT": sinT,
            "binF": np.ascontiguousarray(binF_r).astype(bf),
            "binN": np.ascontiguousarray(mN).astype(bf),
        })
    return in_maps


def _run(inputs, trace=False, **kwargs):
    if "nc" not in _CACHE:
        _CACHE["nc"] = _build()
    nc = _CACHE["nc"]
    in_maps = _host_prep(**inputs)
    res = run_bass_kernel_spmd(
        nc, in_maps, core_ids=list(range(NCORES)), trace=trace, **kwargs)
    outT = np.empty((HID, S), dtype=np.float32)
    for r in range(NCORES):
        outT[r * HPC * P:(r + 1) * HPC * P] = \
            np.asarray(res.results[r]["out"], dtype=np.float32)
    full = np.ascontiguousarray(outT.T).reshape(B, S, HID)
    return full, res


def kernel(**inputs):
    full, _ = _run(inputs, trace=False)
    return full

